# revision 8
# baseline (speedup 1.0000x reference)
"""Trainium2 Bass kernel for nn_AbstractionLayer (gnn_message_passing).

Math (per batch element b):
  w = 1 - clip(gammas,0,1)                                   [R,J,L]
  nmatch[b,rj,i] = -( c0[rj] + sum_l A[rj,l] f[b,i,l] + sum_l W[rj,l] f^2 )
     with c0 = sum_l w*t^2, A = -2*w*t, W = w   (signs folded host-side)
  e = exp(nmatch); attn = e / sum_i e
  selected[b,rj,l] = sum_i attn * f[b,i,l]
  out[b,r,lo] = sum_{j,l} C[r,lo,j,l]*selected[b,(r,j),l] + D[r,lo]
     with C = head_W @ body_W (v contracted), D = head_W@sum_j body_b + head_b

Sharding: pure data parallel over 8 NeuronCores along batch.
Device layout: batch on SBUF partitions, per-element values in the free dim.
DVE does match/products/reduces (bf16 2x mode for the big elementwise stages),
ACT does squares and the 144 exps per element.
"""

import os
import sys

for _p in ("/opt/trn_rl_repo", "/root/.axon_site/_ro/trn_rl_repo"):
    if os.path.isdir(_p) and _p not in sys.path:
        sys.path.insert(0, _p)

import numpy as np

B = 524288
I, R, J, L, V = 12, 6, 2, 2, 4
NCORES = 8
BCORE = B // NCORES  # 65536

P = 128          # partitions
NEL = 32         # elements per partition per chunk
CHUNK = P * NEL
NCHUNK = BCORE // CHUNK

_CACHE = {}


def _build(bcore=BCORE, nel=NEL, fast=True):
    import concourse.bacc as bacc
    import concourse.mybir as mybir
    import concourse.tile as tile

    fp32 = mybir.dt.float32
    dmid = mybir.dt.float16 if fast else mybir.dt.float32

    nchunk = bcore // (P * nel)
    assert nchunk * P * nel == bcore

    nc = bacc.Bacc("TRN2", target_bir_lowering=False, debug=False)

    f_dram = nc.dram_tensor("f", [bcore, I, L], fp32, kind="ExternalInput").ap()
    cm_dram = nc.dram_tensor("consts_mid", [P, 720], dmid, kind="ExternalInput").ap()
    cf_dram = nc.dram_tensor("consts_f32", [P, 60], fp32, kind="ExternalInput").ap()
    out_dram = nc.dram_tensor("out", [bcore, R, L], fp32, kind="ExternalOutput").ap()

    f_view = f_dram.rearrange("(c p n) i l -> c p n i l", c=nchunk, p=P, n=nel)
    o_view = out_dram.rearrange("(c p n) r l -> c p n r l", c=nchunk, p=P, n=nel)

    Exp = mybir.ActivationFunctionType.Exp
    Square = mybir.ActivationFunctionType.Square
    AX = mybir.AxisListType.X

    def bc(ap, axes, shape):
        for ax in axes:
            ap = ap.unsqueeze(ax)
        return ap.broadcast_to(shape)

    with tile.TileContext(nc) as tc:
        with (
            tc.tile_pool(name="const", bufs=1) as cpool,
            tc.tile_pool(name="io", bufs=3) as iop,
            tc.tile_pool(name="mid", bufs=2) as midp,
            tc.tile_pool(name="small", bufs=2) as smp,
        ):
            cm = cpool.tile([P, 720], dmid)
            nc.sync.dma_start(out=cm[:, :], in_=cm_dram[:, :])
            cf = cpool.tile([P, 60], fp32)
            nc.sync.dma_start(out=cf[:, :], in_=cf_dram[:, :])

            sh_m = [P, nel, R * J, I]  # [p, n, rj, i]

            def c12(off):
                # [P, 144] (rj,i)-replicated const -> [P, n, rj, i], innermost stride 1
                a = cm[:, off * 144 : (off + 1) * 144].rearrange(
                    "p (rj i) -> p rj i", rj=R * J)
                return bc(a, [1], sh_m)

            for c in range(nchunk):
                f = iop.tile([P, nel, I, L], fp32, tag="f")
                nc.sync.dma_start(out=f[:, :, :, :], in_=f_view[c])

                # deinterleaved bf16 copies of f (unit innermost stride)
                f0b = midp.tile([P, nel, I], dmid, tag="f0b")
                f1b = midp.tile([P, nel, I], dmid, tag="f1b")
                nc.vector.tensor_copy(f0b[...], f[:, :, :, 0])
                nc.vector.tensor_copy(f1b[...], f[:, :, :, 1])
                # squares on ACT
                q0b = midp.tile([P, nel, I], dmid, tag="q0b")
                q1b = midp.tile([P, nel, I], dmid, tag="q1b")
                nc.scalar.activation(q0b[...], f[:, :, :, 0], Square)
                nc.scalar.activation(q1b[...], f[:, :, :, 1], Square)

                f0 = bc(f0b[:, :, :], [2], sh_m)
                f1 = bc(f1b[:, :, :], [2], sh_m)
                q0 = bc(q0b[:, :, :], [2], sh_m)
                q1 = bc(q1b[:, :, :], [2], sh_m)

                # ---- nm = nA0*f0 + nA1*f1 + nW0*q0 + nW1*q1 + nc0
                t1 = midp.tile(sh_m, dmid, tag="t1")
                t2 = midp.tile(sh_m, dmid, tag="t2")
                nc.vector.tensor_mul(t1[...], f0, c12(0))
                nc.vector.tensor_mul(t2[...], f1, c12(1))
                nc.vector.tensor_add(t1[...], t1[...], t2[...])
                nc.vector.tensor_mul(t2[...], q0, c12(2))
                nc.vector.tensor_add(t1[...], t1[...], t2[...])
                nc.vector.tensor_mul(t2[...], q1, c12(3))
                nc.vector.tensor_add(t1[...], t1[...], t2[...])
                nm = midp.tile(sh_m, dmid, tag="nm")
                nc.vector.tensor_add(nm[...], t1[...], c12(4))

                # ---- e = exp(nm) on ACT
                e = midp.tile(sh_m, dmid, tag="e")
                nc.scalar.activation(e[...], nm[...], Exp)

                # ---- Z and numerators
                # pairwise-add trees (bf16 2x mode) instead of 1x tensor_reduce:
                # 12 -> 6 (bf16) -> 3 (bf16) -> 1 (two fp32 adds)
                def itree(src_ap, out_fp32, tag):
                    h6 = midp.tile([P, nel, R * J, 6], dmid, tag="h6")
                    nc.vector.tensor_add(h6[...], src_ap[:, :, :, 0:6], src_ap[:, :, :, 6:12])
                    h3 = midp.tile([P, nel, R * J, 3], dmid, tag="h3")
                    nc.vector.tensor_add(h3[...], h6[:, :, :, 0:3], h6[:, :, :, 3:6])
                    t = smp.tile([P, nel, R * J], fp32, tag="htmp")
                    nc.vector.tensor_add(t[...], h3[:, :, :, 0], h3[:, :, :, 1])
                    nc.vector.tensor_add(out_fp32[...], t[...], h3[:, :, :, 2])

                Zt = smp.tile([P, nel, R * J], fp32, tag="Z")
                itree(e, Zt, "ze")

                p0 = midp.tile(sh_m, dmid, tag="p0")
                p1 = midp.tile(sh_m, dmid, tag="p1")
                nc.vector.tensor_mul(p0[...], e[...], f0)
                nc.vector.tensor_mul(p1[...], e[...], f1)
                n0 = smp.tile([P, nel, R * J], fp32, tag="n0")
                n1 = smp.tile([P, nel, R * J], fp32, tag="n1")
                itree(p0, n0, "n0")
                itree(p1, n1, "n1")

                # ---- selected = numer / Z
                rz = smp.tile([P, nel, R * J], fp32, tag="rz")
                nc.vector.reciprocal(rz[...], Zt[...])
                s0 = smp.tile([P, nel, R * J], fp32, tag="s0")
                s1 = smp.tile([P, nel, R * J], fp32, tag="s1")
                nc.vector.tensor_mul(s0[...], n0[...], rz[...])
                nc.vector.tensor_mul(s1[...], n1[...], rz[...])

                # ---- out[r,lo] = sum_{j,l} C[r,lo,j,l]*sel[(r,j),l] + D[r,lo]
                sh_o = [P, nel, R, L, J]  # [p, n, r, lo, j]
                s0b = bc(s0[:, :, :].rearrange("p n (r j) -> p n r j", r=R), [3], sh_o)
                s1b = bc(s1[:, :, :].rearrange("p n (r j) -> p n r j", r=R), [3], sh_o)
                C0 = bc(cf[:, 0:24].rearrange("p (r lo j) -> p r lo j", r=R, lo=L), [1], sh_o)
                C1 = bc(cf[:, 24:48].rearrange("p (r lo j) -> p r lo j", r=R, lo=L), [1], sh_o)
                u0 = smp.tile(sh_o, fp32, tag="u0")
                u1 = smp.tile(sh_o, fp32, tag="u1")
                nc.vector.tensor_mul(u0[...], s0b, C0)
                nc.vector.tensor_mul(u1[...], s1b, C1)
                nc.vector.tensor_add(u0[...], u0[...], u1[...])

                ot = iop.tile([P, nel, R, L], fp32, tag="ot")
                nc.vector.tensor_add(ot[...], u0[:, :, :, :, 0], u0[:, :, :, :, 1])
                D = bc(cf[:, 48:60].rearrange("p (r lo) -> p r lo", r=R), [1], [P, nel, R, L])
                nc.vector.tensor_add(ot[...], ot[...], D)

                nc.sync.dma_start(out=o_view[c], in_=ot[:, :, :, :])

    nc.compile()
    return nc


def _host_consts(templates, gammas, body_W, body_b, head_W, head_b):
    t = np.asarray(templates, np.float32).reshape(R * J, L)
    g = np.clip(np.asarray(gammas, np.float32).reshape(R * J, L), 0.0, 1.0)
    w = 1.0 - g
    nA = 2.0 * w * t
    nW = -w
    nc0 = -(w * t * t).sum(-1)
    hW = np.asarray(head_W, np.float32)   # [R, L, V]
    bW = np.asarray(body_W, np.float32)   # [R, J, V, L]
    C = np.einsum("rov,rjvl->rojl", hW, bW)
    D = np.einsum("rov,rv->ro", hW, np.asarray(body_b, np.float32).sum(1)) + np.asarray(
        head_b, np.float32
    )
    cmid = np.zeros((P, 720), np.float32)
    for k, vec in enumerate([nA[:, 0], nA[:, 1], nW[:, 0], nW[:, 1], nc0]):
        cmid[:, k * 144 : (k + 1) * 144] = np.repeat(vec, I)
    cf32 = np.zeros((P, 60), np.float32)
    cf32[:, 0:24] = C[:, :, :, 0].reshape(-1)
    cf32[:, 24:48] = C[:, :, :, 1].reshape(-1)
    cf32[:, 48:60] = D.reshape(-1)
    return cmid, cf32


def kernel(**inputs):
    try:
        from concourse.bass_utils import run_bass_kernel_spmd
    except ImportError:
        from bass_utils import run_bass_kernel_spmd

    f = np.ascontiguousarray(np.asarray(inputs["concrete_features"], np.float32))
    cmid, cf32 = _host_consts(
        inputs["templates"], inputs["gammas"], inputs["body_W"], inputs["body_b"],
        inputs["head_W"], inputs["head_b"],
    )

    if "nc" not in _CACHE:
        _CACHE["nc"] = _build()
    nc = _CACHE["nc"]

    cmid_cast = cmid.astype(np.float16)
    in_maps = [
        {"f": f[c * BCORE : (c + 1) * BCORE], "consts_mid": cmid_cast, "consts_f32": cf32}
        for c in range(NCORES)
    ]
    res = run_bass_kernel_spmd(nc, in_maps, core_ids=list(range(NCORES)))
    outs = [np.asarray(res.results[c]["out"]) for c in range(NCORES)]
    return np.concatenate(outs, axis=0)


# revision 9
# speedup vs baseline: 1.0670x; 1.0670x over previous
"""Trainium2 Bass kernel for nn_AbstractionLayer (gnn_message_passing).

Math (per batch element b):
  w = 1 - clip(gammas,0,1)                                   [R,J,L]
  nmatch[b,rj,i] = -( c0[rj] + sum_l A[rj,l] f[b,i,l] + sum_l W[rj,l] f^2 )
     with c0 = sum_l w*t^2, A = -2*w*t, W = w   (signs folded host-side)
  e = exp(nmatch); attn = e / sum_i e
  selected[b,rj,l] = sum_i attn * f[b,i,l]
  out[b,r,lo] = sum_{j,l} C[r,lo,j,l]*selected[b,(r,j),l] + D[r,lo]
     with C = head_W @ body_W (v contracted), D = head_W@sum_j body_b + head_b

Sharding: pure data parallel over 8 NeuronCores along batch.
Device layout: batch on SBUF partitions, per-element values in the free dim.
DVE does match/products/reduces (bf16 2x mode for the big elementwise stages),
ACT does squares and the 144 exps per element.
"""

import os
import sys

for _p in ("/opt/trn_rl_repo", "/root/.axon_site/_ro/trn_rl_repo"):
    if os.path.isdir(_p) and _p not in sys.path:
        sys.path.insert(0, _p)

import numpy as np

B = 524288
I, R, J, L, V = 12, 6, 2, 2, 4
NCORES = 8
BCORE = B // NCORES  # 65536

P = 128          # partitions
NEL = 32         # elements per partition per chunk
CHUNK = P * NEL
NCHUNK = BCORE // CHUNK

_CACHE = {}


def _build(bcore=BCORE, nel=NEL, fast=True):
    import concourse.bacc as bacc
    import concourse.mybir as mybir
    import concourse.tile as tile

    fp32 = mybir.dt.float32
    dmid = mybir.dt.float16 if fast else mybir.dt.float32

    nchunk = bcore // (P * nel)
    assert nchunk * P * nel == bcore

    nc = bacc.Bacc("TRN2", target_bir_lowering=False, debug=False)

    f_dram = nc.dram_tensor("f", [bcore, I, L], fp32, kind="ExternalInput").ap()
    cm_dram = nc.dram_tensor("consts_mid", [P, 720], dmid, kind="ExternalInput").ap()
    cf_dram = nc.dram_tensor("consts_f32", [P, 60], fp32, kind="ExternalInput").ap()
    out_dram = nc.dram_tensor("out", [bcore, R, L], fp32, kind="ExternalOutput").ap()

    f_view = f_dram.rearrange("(c p n) i l -> c p n i l", c=nchunk, p=P, n=nel)
    o_view = out_dram.rearrange("(c p n) r l -> c p n r l", c=nchunk, p=P, n=nel)

    Exp = mybir.ActivationFunctionType.Exp
    Square = mybir.ActivationFunctionType.Square
    AX = mybir.AxisListType.X

    def bc(ap, axes, shape):
        for ax in axes:
            ap = ap.unsqueeze(ax)
        return ap.broadcast_to(shape)

    with tile.TileContext(nc) as tc:
        with (
            tc.tile_pool(name="const", bufs=1) as cpool,
            tc.tile_pool(name="io", bufs=3) as iop,
            tc.tile_pool(name="mid", bufs=2) as midp,
            tc.tile_pool(name="small", bufs=2) as smp,
        ):
            cm = cpool.tile([P, 720], dmid)
            nc.sync.dma_start(out=cm[:, :], in_=cm_dram[:, :])
            cf = cpool.tile([P, 60], fp32)
            nc.sync.dma_start(out=cf[:, :], in_=cf_dram[:, :])

            sh_m = [P, nel, R * J, I]  # [p, n, rj, i]

            def c12(off):
                # [P, 144] (rj,i)-replicated const -> [P, n, rj, i], innermost stride 1
                a = cm[:, off * 144 : (off + 1) * 144].rearrange(
                    "p (rj i) -> p rj i", rj=R * J)
                return bc(a, [1], sh_m)

            for c in range(nchunk):
                f = iop.tile([P, nel, I, L], fp32, tag="f")
                nc.sync.dma_start(out=f[:, :, :, :], in_=f_view[c])

                # deinterleaved bf16 copies of f (unit innermost stride)
                f0b = midp.tile([P, nel, I], dmid, tag="f0b")
                f1b = midp.tile([P, nel, I], dmid, tag="f1b")
                nc.vector.tensor_copy(f0b[...], f[:, :, :, 0])
                nc.vector.tensor_copy(f1b[...], f[:, :, :, 1])
                # squares on ACT
                q0b = midp.tile([P, nel, I], dmid, tag="q0b")
                q1b = midp.tile([P, nel, I], dmid, tag="q1b")
                nc.scalar.activation(q0b[...], f[:, :, :, 0], Square)
                nc.scalar.activation(q1b[...], f[:, :, :, 1], Square)

                f0 = bc(f0b[:, :, :], [2], sh_m)
                f1 = bc(f1b[:, :, :], [2], sh_m)
                q0 = bc(q0b[:, :, :], [2], sh_m)
                q1 = bc(q1b[:, :, :], [2], sh_m)

                # ---- nm = nA0*f0 + nA1*f1 + nW0*q0 + nW1*q1 + nc0
                t1 = midp.tile(sh_m, dmid, tag="t1")
                t2 = midp.tile(sh_m, dmid, tag="t2")
                nc.vector.tensor_mul(t1[...], f0, c12(0))
                nc.vector.tensor_mul(t2[...], f1, c12(1))
                nc.vector.tensor_add(t1[...], t1[...], t2[...])
                nc.vector.tensor_mul(t2[...], q0, c12(2))
                nc.vector.tensor_add(t1[...], t1[...], t2[...])
                nc.vector.tensor_mul(t2[...], q1, c12(3))
                nc.vector.tensor_add(t1[...], t1[...], t2[...])
                # note: the -c0[rj] term is omitted — a per-(rj) constant
                # factor exp(-c0) cancels between numerator and Z in the
                # softmax-weighted average, so nm only needs the f/q terms.

                # ---- e = exp(nm) on ACT
                e = midp.tile(sh_m, dmid, tag="e")
                nc.scalar.activation(e[...], t1[...], Exp)

                # ---- Z and numerators
                # pairwise-add trees (bf16 2x mode) instead of 1x tensor_reduce:
                # 12 -> 6 (bf16) -> 3 (bf16) -> 1 (two fp32 adds)
                def itree(src_ap, out_fp32, tag):
                    h6 = midp.tile([P, nel, R * J, 6], dmid, tag="h6")
                    nc.vector.tensor_add(h6[...], src_ap[:, :, :, 0:6], src_ap[:, :, :, 6:12])
                    h3 = midp.tile([P, nel, R * J, 3], dmid, tag="h3")
                    nc.vector.tensor_add(h3[...], h6[:, :, :, 0:3], h6[:, :, :, 3:6])
                    t = smp.tile([P, nel, R * J], fp32, tag="htmp")
                    nc.vector.tensor_add(t[...], h3[:, :, :, 0], h3[:, :, :, 1])
                    nc.vector.tensor_add(out_fp32[...], t[...], h3[:, :, :, 2])

                Zt = smp.tile([P, nel, R * J], fp32, tag="Z")
                itree(e, Zt, "ze")

                p0 = midp.tile(sh_m, dmid, tag="p0")
                p1 = midp.tile(sh_m, dmid, tag="p1")
                nc.vector.tensor_mul(p0[...], e[...], f0)
                nc.vector.tensor_mul(p1[...], e[...], f1)
                n0 = smp.tile([P, nel, R * J], fp32, tag="n0")
                n1 = smp.tile([P, nel, R * J], fp32, tag="n1")
                itree(p0, n0, "n0")
                itree(p1, n1, "n1")

                # ---- selected = numer / Z
                rz = smp.tile([P, nel, R * J], fp32, tag="rz")
                nc.vector.reciprocal(rz[...], Zt[...])
                s0 = smp.tile([P, nel, R * J], fp32, tag="s0")
                s1 = smp.tile([P, nel, R * J], fp32, tag="s1")
                nc.vector.tensor_mul(s0[...], n0[...], rz[...])
                nc.vector.tensor_mul(s1[...], n1[...], rz[...])

                # ---- out[r,lo] = sum_{j,l} C[r,lo,j,l]*sel[(r,j),l] + D[r,lo]
                sh_o = [P, nel, R, L, J]  # [p, n, r, lo, j]
                s0b = bc(s0[:, :, :].rearrange("p n (r j) -> p n r j", r=R), [3], sh_o)
                s1b = bc(s1[:, :, :].rearrange("p n (r j) -> p n r j", r=R), [3], sh_o)
                C0 = bc(cf[:, 0:24].rearrange("p (r lo j) -> p r lo j", r=R, lo=L), [1], sh_o)
                C1 = bc(cf[:, 24:48].rearrange("p (r lo j) -> p r lo j", r=R, lo=L), [1], sh_o)
                u0 = smp.tile(sh_o, fp32, tag="u0")
                u1 = smp.tile(sh_o, fp32, tag="u1")
                nc.vector.tensor_mul(u0[...], s0b, C0)
                nc.vector.tensor_mul(u1[...], s1b, C1)
                nc.vector.tensor_add(u0[...], u0[...], u1[...])

                ot = iop.tile([P, nel, R, L], fp32, tag="ot")
                nc.vector.tensor_add(ot[...], u0[:, :, :, :, 0], u0[:, :, :, :, 1])
                D = bc(cf[:, 48:60].rearrange("p (r lo) -> p r lo", r=R), [1], [P, nel, R, L])
                nc.vector.tensor_add(ot[...], ot[...], D)

                nc.sync.dma_start(out=o_view[c], in_=ot[:, :, :, :])

    nc.compile()
    return nc


def _host_consts(templates, gammas, body_W, body_b, head_W, head_b):
    t = np.asarray(templates, np.float32).reshape(R * J, L)
    g = np.clip(np.asarray(gammas, np.float32).reshape(R * J, L), 0.0, 1.0)
    w = 1.0 - g
    nA = 2.0 * w * t
    nW = -w
    nc0 = -(w * t * t).sum(-1)
    hW = np.asarray(head_W, np.float32)   # [R, L, V]
    bW = np.asarray(body_W, np.float32)   # [R, J, V, L]
    C = np.einsum("rov,rjvl->rojl", hW, bW)
    D = np.einsum("rov,rv->ro", hW, np.asarray(body_b, np.float32).sum(1)) + np.asarray(
        head_b, np.float32
    )
    cmid = np.zeros((P, 720), np.float32)
    for k, vec in enumerate([nA[:, 0], nA[:, 1], nW[:, 0], nW[:, 1], nc0]):
        cmid[:, k * 144 : (k + 1) * 144] = np.repeat(vec, I)
    cf32 = np.zeros((P, 60), np.float32)
    cf32[:, 0:24] = C[:, :, :, 0].reshape(-1)
    cf32[:, 24:48] = C[:, :, :, 1].reshape(-1)
    cf32[:, 48:60] = D.reshape(-1)
    return cmid, cf32


def kernel(**inputs):
    try:
        from concourse.bass_utils import run_bass_kernel_spmd
    except ImportError:
        from bass_utils import run_bass_kernel_spmd

    f = np.ascontiguousarray(np.asarray(inputs["concrete_features"], np.float32))
    cmid, cf32 = _host_consts(
        inputs["templates"], inputs["gammas"], inputs["body_W"], inputs["body_b"],
        inputs["head_W"], inputs["head_b"],
    )

    if "nc" not in _CACHE:
        _CACHE["nc"] = _build()
    nc = _CACHE["nc"]

    cmid_cast = cmid.astype(np.float16)
    in_maps = [
        {"f": f[c * BCORE : (c + 1) * BCORE], "consts_mid": cmid_cast, "consts_f32": cf32}
        for c in range(NCORES)
    ]
    res = run_bass_kernel_spmd(nc, in_maps, core_ids=list(range(NCORES)))
    outs = [np.asarray(res.results[c]["out"]) for c in range(NCORES)]
    return np.concatenate(outs, axis=0)
